# revision 22
# baseline (speedup 1.0000x reference)
"""Trainium2 Bass kernel for nn_EssentialMatixModule.

Dual-softmax cross-attention (LoFTR-style) + bilinear feature + projection.
Data-parallel over batch across 8 cores; proj output-sharded with chunked
AllGathers of the (bf16) feature matrix overlapping the attention phase.

v2: x pre-scaled by 1/sigma (column broadcast via DMA) so QKV psum
evacuations are pure casts on the scalar engine; E^2 on DVE 2x mode; zc via
DVE pair-sums + 6 matmuls; 1/zr folded onto vpl, 1/zc onto vpc (both 4x
tensor_scalar); single-bank up psum with one batched evacuation; 3-deep
S-psum rotation and interleaved PE emission for a dense matmul stream.
"""

import sys

sys.path.insert(0, "/opt/trn_rl_repo")

from contextlib import ExitStack

import ml_dtypes
import numpy as np

import concourse.bass as bass
import concourse.tile as tile
from concourse import bacc, mybir
from concourse.bass_utils import run_bass_kernel_spmd

B, C, HG, WG = 64, 256, 24, 24
N = HG * WG  # 576
H, HD = 3, 64
F = H * HD  # 192
SCALE = HD**-0.5
EPS = 1e-5
NCORES = 8
BP = B // NCORES  # 8 items per core
NT = [128, 128, 128, 128, 64]  # token tiles (sum=576)
NCH = [(0, 512), (512, 64)]  # free-dim chunks for N=576 psum
DE = 70  # hd + 6 pos dims
PADMH = 4992  # 39*128, per-(map,head) padded feat block
DIMS = 6 * PADMH  # 29952
OS = 512 // NCORES  # 64 output cols per core
F32 = mybir.dt.float32
BF16 = mybir.dt.bfloat16
AX = mybir.ActivationFunctionType
OP = mybir.AluOpType


def _host_prep(ln_w, ln_b, qkv_w, proj_w, proj_b):
    ln_w = ln_w.astype(np.float64)
    ln_b = ln_b.astype(np.float64)
    qw = qkv_w.astype(np.float64)
    Wp = qw * ln_w[None, :]  # [576, C]
    r = Wp.sum(axis=1)  # [576]
    t = qw @ ln_b  # [576]

    # per-side packing: side0 tiles hold [k_h; q_h], side1 [q_h; k_h] so the
    # attention matmul operands always share a partition base
    def col(fsl, scale):
        return np.concatenate([Wp[fsl] * scale, (r[fsl] * scale)[:, None],
                               (t[fsl] * scale)[:, None]], axis=1).T

    wqk = np.zeros((2, C + 2, 3 * 128), np.float32)
    for h in range(H):
        qr = slice(h * HD, (h + 1) * HD)
        kr = slice(F + h * HD, F + (h + 1) * HD)
        qcols = col(qr, SCALE)  # [C+2, 64]
        kcols = col(kr, 1.0)
        wqk[0, :, h * 128 : h * 128 + 64] = kcols
        wqk[0, :, h * 128 + 64 : h * 128 + 128] = qcols
        wqk[1, :, h * 128 : h * 128 + 64] = qcols
        wqk[1, :, h * 128 + 64 : h * 128 + 128] = kcols
    wqk = wqk.astype(ml_dtypes.bfloat16)

    wv = np.zeros((C + 2, F), np.float32)
    wv[:C] = Wp[2 * F :].T
    wv[C] = r[2 * F :]
    wv[C + 1] = t[2 * F :]
    wv = wv.astype(ml_dtypes.bfloat16)

    ys = np.linspace(-1.0, 1.0, HG)
    xs = np.linspace(-1.0, 1.0, WG)
    p3 = np.tile(ys, WG)
    p4 = np.repeat(xs, HG)
    pos = np.stack([p3 * p3, p4 * p4, p3 * p4, p3, p4, np.ones_like(p3)], axis=1)
    pos_pad = np.zeros((640, 6), np.float32)
    pos_pad[:N] = pos

    pwt = np.zeros((DIMS, 512), np.float32)
    for mh in range(6):
        blk = proj_w[:, mh * 4900 : (mh + 1) * 4900]  # [512, 4900]
        pwt[mh * PADMH : mh * PADMH + 4900] = blk.T
    pwt = pwt.astype(ml_dtypes.bfloat16)
    return wqk, wv, pos_pad, pwt


def _build():
    nc = bacc.Bacc()
    x1d = nc.declare_dram_parameter("x1s", [BP, C, N], BF16, isOutput=False)
    x2d = nc.declare_dram_parameter("x2s", [BP, C, N], BF16, isOutput=False)
    wqkd = nc.declare_dram_parameter("wqk", [2, C + 2, 3 * 128], BF16, isOutput=False)
    wvd = nc.declare_dram_parameter("wv", [C + 2, F], BF16, isOutput=False)
    posd = nc.declare_dram_parameter("pos", [640, 6], F32, isOutput=False)
    pwtd = nc.declare_dram_parameter("pwt", [DIMS, OS], BF16, isOutput=False)
    pbd = nc.declare_dram_parameter("pb", [1, OS], F32, isOutput=False)
    outd = nc.declare_dram_parameter("out", [B, OS], F32, isOutput=True)
    # per side: row0 = -mu/sigma, row1 = 1/sigma  (bf16), [2, 2, BP, N]
    statsd = nc.dram_tensor("statsd", [2, 2, BP, N], BF16)
    statsdf = nc.dram_tensor("statsdf", [2, BP, N], F32)
    feat8d = [nc.dram_tensor(f"feat8_{j}", [BP, PADMH], BF16) for j in range(6)]
    featAG = [
        nc.dram_tensor(f"featAG_{j}", [B, PADMH], BF16, addr_space="Shared")
        for j in range(6)
    ]
    xd = [x1d, x2d]

    def bcast_p(sl, p):
        return bass.AP(tensor=sl.tensor, offset=sl.offset, ap=[[0, p]] + list(sl.ap))

    with ExitStack() as ctx:
        tc = ctx.enter_context(tile.TileContext(nc))
        const = ctx.enter_context(tc.tile_pool(name="const", bufs=1))
        xin = ctx.enter_context(tc.tile_pool(name="xin", bufs=6))
        stats = ctx.enter_context(tc.tile_pool(name="stats", bufs=1))
        tmp = ctx.enter_context(tc.tile_pool(name="tmp", bufs=4))
        isbp = ctx.enter_context(tc.tile_pool(name="isbp", bufs=3))
        sb_qk = ctx.enter_context(tc.tile_pool(name="sbqk", bufs=1))
        sb_vp = ctx.enter_context(tc.tile_pool(name="sbvp", bufs=1))
        epool = ctx.enter_context(tc.tile_pool(name="epool", bufs=12))
        e2pool = ctx.enter_context(tc.tile_pool(name="e2pool", bufs=12))
        espool = ctx.enter_context(tc.tile_pool(name="espool", bufs=3))
        zpool = ctx.enter_context(tc.tile_pool(name="zpool", bufs=6))
        upool = ctx.enter_context(tc.tile_pool(name="upool", bufs=12))
        uspool = ctx.enter_context(tc.tile_pool(name="uspool", bufs=4))
        fpool = ctx.enter_context(tc.tile_pool(name="fpool", bufs=4))
        ftpool = ctx.enter_context(tc.tile_pool(name="ftpool", bufs=3))
        opool = ctx.enter_context(tc.tile_pool(name="opool", bufs=2))
        # PSUM: S-pool 3x2 banks, up 1 bank, fps(+proj) 1 bank
        psS = ctx.enter_context(tc.tile_pool(name="psS", bufs=3, space="PSUM"))
        psU = ctx.enter_context(tc.tile_pool(name="psU", bufs=1, space="PSUM"))
        psF = ctx.enter_context(tc.tile_pool(name="psF", bufs=1, space="PSUM"))

        # ---- constants ----
        wqk_sb = [[], []]
        wv_sb = []
        for k, (k0, kw) in enumerate([(0, 128), (128, 128), (256, 2)]):
            for s in range(2):
                wt = const.tile([kw, 3 * 128], BF16, tag=f"wqk{s}_{k}")
                nc.sync.dma_start(out=wt, in_=wqkd[s, k0 : k0 + kw, :])
                wqk_sb[s].append(wt)
            vt = const.tile([kw, F], BF16, tag=f"wv{k}")
            nc.sync.dma_start(out=vt, in_=wvd[k0 : k0 + kw, :])
            wv_sb.append(vt)
        pos_sb = const.tile([128, 5, 6], F32, tag="pos")
        nc.sync.dma_start(out=pos_sb, in_=posd.rearrange("(t p) e -> p t e", p=128))
        ind8_sb = const.tile([128, BP, BP], BF16, tag="ind8")
        nc.vector.memset(ind8_sb, 0.0)
        for i in range(BP):
            nc.vector.memset(ind8_sb[:, i, i : i + 1], 1.0)
        onesb_sb = const.tile([128, 32], BF16, tag="onesb")
        nc.vector.memset(onesb_sb, 1.0)
        epssb = const.tile([32, 1], F32, tag="eps")
        nc.vector.memset(epssb, EPS)
        pb_sb = const.tile([B, OS], F32, tag="pb")
        nc.gpsimd.dma_start(out=pb_sb, in_=bcast_p(pbd[0, :], B))
        zpad = const.tile([1, 552], BF16, tag="zpad")
        nc.vector.memset(zpad, 0.0)
        for j in range(6):
            for ib in range(BP):
                nc.gpsimd.dma_start(
                    out=feat8d[j][ib, 4900:PADMH],
                    in_=bass.AP(tensor=zpad.tensor, offset=zpad.offset, ap=[[1, 1], [1, 92]]),
                )
        # negmu_flat[s]: [2, BP*N] row0 = -mu/sigma (dma later), row1 = ones
        nmf = []
        for s in range(2):
            t_ = const.tile([2, BP, N], BF16, tag=f"nmf{s}")
            nc.vector.memset(t_, 1.0)  # row 0 overwritten by -mu/sigma DMA later
            nmf.append(t_)

        # ---- phase 1a: LN stats (batched per side) ----
        for s in range(2):
            psum_s = psS.tile([128, N], F32, tag="pq", name="psum_s")[0:BP]
            psum_q = psS.tile([128, N], F32, tag="pq", name="psum_q")[0:BP]
            for i in range(BP):
                for k in range(2):
                    xt = xin.tile([128, N], BF16, tag="x")
                    nc.scalar.dma_start(out=xt, in_=xd[s][i, k * 128 : (k + 1) * 128, :])
                    xq = tmp.tile([128, N], BF16, tag="xsq")
                    nc.vector.tensor_mul(xq, xt, xt)
                    st = i == 0 and k == 0
                    for c0, cw in NCH:
                        nc.tensor.matmul(
                            psum_s[:, c0 : c0 + cw], ind8_sb[:, i, :], xt[:, c0 : c0 + cw],
                            start=st, stop=(i == BP - 1 and k == 1),
                        )
                        nc.tensor.matmul(
                            psum_q[:, c0 : c0 + cw], ind8_sb[:, i, :], xq[:, c0 : c0 + cw],
                            start=st, stop=(i == BP - 1 and k == 1),
                        )
            mean = stats.tile([32, N], F32, tag="mean")
            ex2 = stats.tile([32, N], F32, tag="ex2")
            nc.vector.tensor_scalar_mul(mean[:BP], psum_s, 1.0 / C)
            nc.vector.tensor_scalar_mul(ex2[:BP], psum_q, 1.0 / C)
            var = stats.tile([32, N], F32, tag="var")
            nc.vector.scalar_tensor_tensor(
                out=var[:BP], in0=mean[:BP], scalar=-1.0, in1=mean[:BP], op0=OP.mult, op1=OP.mult
            )
            nc.vector.tensor_add(var[:BP], var[:BP], ex2[:BP])
            sig = stats.tile([32, N], F32, tag="sig")
            nc.scalar.activation(out=sig[:BP], in_=var[:BP], func=AX.Sqrt, bias=epssb[:BP])
            isv = stats.tile([32, N], F32, tag="isv")
            nc.vector.reciprocal(isv[:BP], sig[:BP])
            # -mu/sigma and 1/sigma, bf16
            nmu_is = stats.tile([32, N], BF16, tag="nmu_is")
            nc.vector.scalar_tensor_tensor(
                out=nmu_is[:BP], in0=mean[:BP], scalar=-1.0, in1=isv[:BP],
                op0=OP.mult, op1=OP.mult,
            )
            nc.sync.dma_start(out=statsd[s, 0], in_=nmu_is[:BP])
            nc.sync.dma_start(out=statsdf[s], in_=isv[:BP])
            # flat [-mu/sigma] row at partition 0 (for k=2 matmul operands)
            nc.sync.dma_start(
                out=nmf[s][0:1, :, :].rearrange("p b n -> p (b n)"),
                in_=bcast_p(statsd[s, 0].rearrange("b n -> (b n)"), 1),
            )

        # ---- phase 1b: QKV for all items (x pre-scaled by 1/sigma) ----
        qs = {}
        ks = {}
        vp = {}
        for i in range(BP):
            for s in range(2):
                # broadcast 1/sigma row of item i across 128 partitions via DMA
                isb = isbp.tile([128, N], F32, tag="isb")
                nc.gpsimd.dma_start(out=isb, in_=bcast_p(statsdf[s, i, :], 128))
                xt0 = xin.tile([128, N], BF16, tag="x")
                nc.sync.dma_start(out=xt0, in_=xd[s][i, 0:128, :])
                xt1 = xin.tile([128, N], BF16, tag="x")
                nc.sync.dma_start(out=xt1, in_=xd[s][i, 128:256, :])
                nc.vector.tensor_mul(xt0, xt0, isb)
                nc.vector.tensor_mul(xt1, xt1, isb)
                rhs3 = [xt0, xt1, nmf[s][:, i, :]]
                # per-side packed [k;q] (s=0) / [q;k] (s=1) tiles
                for h in range(H):
                    pq = psS.tile([128, N], F32, tag="pq")
                    for k in range(3):
                        for c0, cw in NCH:
                            nc.tensor.matmul(
                                pq[:, c0 : c0 + cw],
                                wqk_sb[s][k][:, h * 128 : (h + 1) * 128],
                                rhs3[k][:, c0 : c0 + cw],
                                start=(k == 0), stop=(k == 2),
                            )
                    qk = sb_qk.tile([128, N], BF16, tag=f"qk{i}_{s}_{h}")
                    nc.scalar.activation(out=qk, in_=pq, func=AX.Copy)
                    if s == 0:
                        ks[i, s, h] = qk[0:64, :]
                        qs[i, s, h] = qk[64:128, :]
                    else:
                        qs[i, s, h] = qk[0:64, :]
                        ks[i, s, h] = qk[64:128, :]
                for nt in range(5):
                    w = NT[nt]
                    n0 = nt * 128
                    pv = psS.tile([128, N], F32, tag="pq")
                    for k in range(2):
                        nc.tensor.matmul(
                            pv[:w, 0:F],
                            rhs3[k][:, n0 : n0 + w],
                            wv_sb[k],
                            start=(k == 0), stop=False,
                        )
                    nc.tensor.matmul(
                        pv[:w, 0:F],
                        nmf[s][:, i, n0 : n0 + w],
                        wv_sb[2],
                        start=False, stop=True,
                    )
                    vt = sb_vp.tile([128, 3, 72], BF16, tag=f"vp{i}_{s}_{nt}")
                    nc.vector.tensor_copy(
                        out=vt[:w, :, 0:64],
                        in_=pv[:w, 0:F].rearrange("p (a b) -> p a b", b=64),
                    )
                    ps = pos_sb[:w, nt, :]
                    nc.vector.tensor_copy(
                        out=vt[:w, :, 64:70],
                        in_=bass.AP(tensor=ps.tensor, offset=ps.offset,
                                    ap=[ps.ap[0], [0, 3], ps.ap[-1]]),
                    )
                    vp[i, s, nt] = vt

        # ---- phase 2: attention, software-pipelined; AllGather + proj ----
        oacc = opool.tile([B, OS], F32, tag="oacc")
        nc.vector.memset(oacc, 0.0)

        def emit_proj(mh):
            GSZ = 13
            for g0 in range(0, 39, GSZ):
                ft = ftpool.tile([128, GSZ, B], BF16, tag="ft")
                nc.sync.dma_start_transpose(
                    out=ft, in_=featAG[mh][:, g0 * 128 : (g0 + GSZ) * 128]
                )
                pw = ftpool.tile([128, GSZ, OS], BF16, tag="pw")
                nc.gpsimd.dma_start(
                    out=pw,
                    in_=pwtd[mh * PADMH + g0 * 128 : mh * PADMH + (g0 + GSZ) * 128, :]
                    .rearrange("(j p) o -> p j o", p=128),
                )
                opsum = psF.tile([64, OS], F32, tag="fps")
                for j in range(GSZ):
                    nc.tensor.matmul(
                        opsum, ft[:, j, :], pw[:, j, :],
                        start=(j == 0), stop=(j == GSZ - 1),
                    )
                nc.vector.tensor_add(oacc, oacc, opsum)

        def emit_gather(j):
            nc.gpsimd.collective_compute(
                "AllGather",
                OP.bypass,
                replica_groups=[list(range(NCORES))],
                ins=[feat8d[j][:]],
                outs=[featAG[j][:]],
            )

        # block order: (h, i) pairs with m inner so adjacent blocks use
        # disjoint PE row halves (m=0: rows 0-63, m=1: rows 64-127)
        blocks = []
        for h in range(H):
            for i in range(BP):
                for m in range(2):
                    blocks.append((m, h, i))
        NB = len(blocks)
        NP = NB // 2

        # per-block live state
        S_ps = {}     # (bi, nt) -> S psum tile
        E_t = {}      # (bi, nt) -> E sbuf bf16
        E2_t = {}     # (bi, nt) -> E^2 sbuf bf16
        ES_t = {}     # (bi, pair) -> E pair sums
        zr5 = {}      # bi -> [128, 8] f32 accum of exp row sums
        rzr5 = {}     # bi -> reciprocal
        zc_ps = {}    # bi -> zc psum [32, N]
        vpl_t = {}    # (bi, nt) -> vp scaled by 1/zr
        vpc_t = {}    # (bi, mc) -> vp scaled by 1/zc
        rzc_t = {}    # bi -> [128, 5] per-partition 1/zc
        up_ps = {}    # bi -> up psum [128, 5, 72]
        us_t = {}     # bi -> up evac sbuf
        fps_ps = {}   # bi -> fps psum [70, 72]

        proj_at = {3: [0, 1], 5: [2, 3]}
        gather_at = {1: [0, 1], 3: [2, 3], 4: [4]}

        def qk_chunk(bi, nt, ci):
            # chunk-split QK emission: A/B blocks sit on disjoint PE row
            # halves, so ordering [A512, B512, A64, B64] lets each small
            # chunk's LDWEIGHTS pull ahead under the other block's stream
            m, h, i = blocks[bi]
            qside, vside = 1 - m, m
            w = NT[nt]
            n0 = nt * 128
            if ci == 0:
                pa = psS.tile([128, N], F32, tag="pq", name="pa")
                S_ps[bi, nt] = pa
            pa = S_ps[bi, nt]
            c0, cw = NCH[ci]
            nc.tensor.matmul(
                pa[:w, c0 : c0 + cw],
                qs[i, qside, h][:, n0 : n0 + w],
                ks[i, vside, h][:, c0 : c0 + cw],
                start=True, stop=True,
            )

        def qk_pair(ba, bb, nt):
            qk_chunk(ba, nt, 0)
            qk_chunk(bb, nt, 0)
            qk_chunk(ba, nt, 1)
            qk_chunk(bb, nt, 1)

        def exp_act(bi, nt):
            w = NT[nt]
            if nt == 0:
                zr5[bi] = zpool.tile([128, 8], F32, tag="zr5", name="zr5")
            et = epool.tile([128, N], BF16, tag="E", name="et")
            E_t[bi, nt] = et
            nc.scalar.activation(
                out=et[:w], in_=S_ps.pop((bi, nt))[:w], func=AX.Exp,
                accum_out=zr5[bi][:w, nt : nt + 1],
            )

        def e2_mul(bi, nt):
            w = NT[nt]
            e2 = e2pool.tile([128, N], BF16, tag="E2", name="e2")
            E2_t[bi, nt] = e2
            nc.vector.tensor_mul(e2[:w], E_t[bi, nt][:w], E_t[bi, nt][:w])

        def esum(bi):
            # full tree-reduce of the 5 E tiles to one [128, N] tile so the
            # zc colsum matmul streams 576 rows instead of 1728
            es0 = espool.tile([128, N], BF16, tag="es0", name="es0", bufs=2)
            nc.vector.tensor_add(es0, E_t[bi, 0], E_t[bi, 1])
            es1 = espool.tile([128, N], BF16, tag="es1", name="es1", bufs=2)
            nc.gpsimd.tensor_add(es1, E_t[bi, 2], E_t[bi, 3])
            es = espool.tile([128, N], BF16, tag="es2", name="es2")
            nc.vector.tensor_add(es, es0, es1)
            nc.vector.tensor_add(es[0:64], es[0:64], E_t[bi, 4][0:64])
            ES_t[bi] = es

        def rzr(bi):
            r = zpool.tile([128, 8], F32, tag="rzr5", name="rzr5")
            rzr5[bi] = r
            nc.vector.reciprocal(r[:, 0:5], zr5[bi][:, 0:5])

        def vpl_scale(bi, nt):
            m, h, i = blocks[bi]
            vside = m
            w = NT[nt]
            vpl = upool.tile([128, 72], BF16, tag="vpl", name="vpl")
            vpl_t[bi, nt] = vpl
            nc.vector.tensor_scalar_mul(
                vpl[:w, 0:70], vp[i, vside, nt][:w, h, 0:70],
                rzr5[bi][:w, nt : nt + 1],
            )

        def zc_mms(bi):
            zcp = psS.tile([128, N], F32, tag="pq", name="zcp")[0:32]
            zc_ps[bi] = zcp
            src_ = ES_t[bi]
            for c0, cw in NCH:
                nc.tensor.matmul(
                    zcp[:, c0 : c0 + cw], onesb_sb[:, :], src_[:, c0 : c0 + cw],
                    start=True, stop=True,
                )

        def rzc(bi):
            zt = tmp.tile([32, 18, 32], F32, tag="zt")
            nc.vector.transpose(
                out=zt, in_=zc_ps.pop(bi).rearrange("p (g q) -> p g q", q=32)
            )
            rz32 = zpool.tile([32, 18], F32, tag="rz32")
            nc.vector.reciprocal(rz32, zt[:, :, 0])
            r = zpool.tile([128, 5], F32, tag="rzc", name="rzc")
            rzc_t[bi] = r
            for a in range(4):
                ng = 5 if a < 2 else 4
                nc.vector.tensor_copy(
                    out=r[32 * a : 32 * a + 32, 0:ng],
                    in_=rz32[:, a : 18 : 4],
                )

        def vpc_scale(bi, mc):
            m, h, i = blocks[bi]
            vside = m
            w2 = NT[mc]
            vpc = upool.tile([128, 72], BF16, tag="vpc", name="vpc")
            vpc_t[bi, mc] = vpc
            nc.vector.tensor_scalar_mul(
                vpc[:w2, 0:70], vp[i, vside, mc][:w2, h, 0:70],
                rzc_t[bi][:w2, mc : mc + 1],
            )

        def up_mms(bi, nt):
            w = NT[nt]
            if nt == 0:
                up_ps[bi] = psU.tile([128, 5, 72], F32, tag="up", name="up")
            upt = up_ps[bi]
            e2 = E2_t[bi, nt]
            vpl = vpl_t[bi, nt]
            # single accumulation group for the whole bank: start=True clears
            # has_written for the entire 2KB bank, so per-mc groups would wipe
            # each other's partials. One group + per-element has_written works.
            for mc in range(5):
                w2 = NT[mc]
                nc.tensor.matmul(
                    upt[:w2, mc, 0:70],
                    e2[:w, mc * 128 : mc * 128 + w2],
                    vpl[:w, 0:70],
                    start=(nt == 0 and mc == 0), stop=(nt == 4 and mc == 4),
                    skip_group_check=True,
                )

        def up_evac(bi, on_act):
            us = uspool.tile([128, 5, 72], BF16, tag="us", name="us")
            us_t[bi] = us
            upt = up_ps.pop(bi)
            # read only matmul-written psum regions (mc 0-3 full, mc 4 rows 0-63)
            if on_act:
                nc.scalar.activation(out=us[:, 0:4, 0:70], in_=upt[:, 0:4, 0:70], func=AX.Copy)
                nc.scalar.activation(out=us[0:64, 4, 0:70], in_=upt[0:64, 4, 0:70], func=AX.Copy)
            else:
                nc.vector.tensor_copy(out=us[:, 0:4, 0:70], in_=upt[:, 0:4, 0:70])
                nc.vector.tensor_copy(out=us[0:64, 4, 0:70], in_=upt[0:64, 4, 0:70])
            # drop E2/vpl refs
            for nt in range(5):
                E2_t.pop((bi, nt))
                vpl_t.pop((bi, nt))

        def fps_mms(bi):
            fp = psF.tile([70, 72], F32, tag="fps", name="fps")
            fps_ps[bi] = fp
            us = us_t.pop(bi)
            for mc in range(5):
                w2 = NT[mc]
                nc.tensor.matmul(
                    fp[0:70, 0:70],
                    us[:w2, mc, 0:70],
                    vpc_t.pop((bi, mc))[:w2, 0:70],
                    start=(mc == 0), stop=(mc == 4),
                )

        def fb_out(bi):
            m, h, i = blocks[bi]
            mh = m * 3 + h
            fb = fpool.tile([70, 70], BF16, tag="fb", name="fb")
            nc.vector.tensor_copy(out=fb, in_=fps_ps.pop(bi)[0:70, 0:70])
            nc.sync.dma_start(
                out=feat8d[mh][i, 0:4900].rearrange("(d e) -> d e", e=70),
                in_=fb,
            )

        def drop_e(bi):
            for nt in range(5):
                E_t.pop((bi, nt))
            ES_t.pop(bi)
            zr5.pop(bi)
            rzr5.pop(bi)
            rzc_t.pop(bi)

        # prologue: QK + exp for pair 0 (blocks 0, 1), interleaved A/B
        for nt in range(5):
            qk_pair(0, 1, nt)
        for nt in range(5):
            exp_act(0, nt)
            exp_act(1, nt)

        # gather j after the fb of its (h, i=7) pair lands (pair-iter 8h+7+2);
        # proj j a few pair-iters after its gather
        gather_sched = {10: [0, 3], 18: [1, 4]}
        proj_sched = {13: [0], 16: [3], 21: [1], 24: [4]}

        # steady state over pairs: iteration p emits QK(p), math(p-1), out(p-2)
        evac_q = []
        for p in range(1, NP + 3):
            A, Bb = 2 * p, 2 * p + 1          # current pair blocks
            A1, B1 = A - 2, Bb - 2            # math stage
            A2, B2 = A - 6, Bb - 6            # out stage (after up_evac lag)
            for pj in proj_sched.get(p, []):
                emit_proj(pj)
            # DVE: evacuate previous pair's up psums first (frees psU bank),
            # then the math-front for (A1, B1)
            for X in evac_q:
                up_evac(X, on_act=False)
                for mc in range(5):
                    vpc_scale(X, mc)
            evac_q = []
            if A1 < NB:
                for X in (A1, B1):
                    rzr(X)
                    for nt in range(5):
                        e2_mul(X, nt)
                        vpl_scale(X, nt)
                    esum(X)
            # PE: QK pairs interleaved with fps of the out stage
            if p < NP:
                qk_pair(A, Bb, 0)
            if 0 <= A2 < NB:
                fps_mms(A2)
            if p < NP:
                qk_pair(A, Bb, 1)
            if 0 <= B2 < NB:
                fps_mms(B2)
            if p < NP:
                for nt in range(2, 5):
                    qk_pair(A, Bb, nt)
            if A1 < NB:
                for nt in range(5):
                    up_mms(A1, nt)
                zc_mms(A1)
                for nt in range(5):
                    up_mms(B1, nt)
                zc_mms(B1)
                evac_q = [A1, B1]
                    # ACT: exps for pair p in S-pool allocation order
            if p < NP:
                for nt in range(5):
                    exp_act(A, nt)
                    exp_act(Bb, nt)
            if A1 < NB:
                rzc(A1)
                rzc(B1)
            if 0 <= A2 < NB:
                fb_out(A2)
                fb_out(B2)
                drop_e(A2)
                drop_e(B2)
            for j in gather_sched.get(p, []):
                emit_gather(j)

        emit_gather(2)
        emit_gather(5)
        emit_proj(2)
        emit_proj(5)
        osb = opool.tile([B, OS], F32, tag="osb")
        nc.vector.tensor_add(osb, oacc, pb_sb)
        nc.vector.tensor_scalar_max(osb, osb, 0.0)
        nc.sync.dma_start(out=outd[:], in_=osb)

    nc.compile()
    return nc


def kernel(x1, x2, ln_w, ln_b, qkv_w, proj_w, proj_b):
    wqk, wv, pos_pad, pwt = _host_prep(ln_w, ln_b, qkv_w, proj_w, proj_b)
    xs1 = np.ascontiguousarray(x1.reshape(B, C, N)).astype(ml_dtypes.bfloat16)
    xs2 = np.ascontiguousarray(x2.reshape(B, C, N)).astype(ml_dtypes.bfloat16)
    nc = _build()
    in_maps = []
    for r in range(NCORES):
        in_maps.append(
            {
                "x1s": xs1[r * BP : (r + 1) * BP],
                "x2s": xs2[r * BP : (r + 1) * BP],
                "wqk": wqk,
                "wv": wv,
                "pos": pos_pad,
                "pwt": np.ascontiguousarray(pwt[:, r * OS : (r + 1) * OS]),
                "pb": np.ascontiguousarray(proj_b[None, r * OS : (r + 1) * OS]).astype(np.float32),
            }
        )
    import os

    trace = bool(os.environ.get("BASS_TRACE"))
    res = run_bass_kernel_spmd(nc, in_maps, core_ids=list(range(NCORES)), trace=trace)
    if res.exec_time_ns is not None:
        print(f"HW exec time: {res.exec_time_ns} ns")
    if res.instructions_and_trace:
        print("trace path:", res.instructions_and_trace[1])
    out = np.concatenate([res.results[r]["out"] for r in range(NCORES)], axis=1)
    return out.astype(np.float32)


if __name__ == "__main__":
    rng = np.random.default_rng(0)
    ins = {
        "x1": rng.standard_normal((B, C, HG, WG), dtype=np.float32),
        "x2": rng.standard_normal((B, C, HG, WG), dtype=np.float32),
        "ln_w": np.ones(C, np.float32),
        "ln_b": np.zeros(C, np.float32),
        "qkv_w": (rng.standard_normal((3 * F, C)) * C**-0.5).astype(np.float32),
        "proj_w": (rng.standard_normal((512, 6 * 4900)) * (6 * 4900) ** -0.5).astype(np.float32),
        "proj_b": np.zeros(512, np.float32),
    }
    print(kernel(**ins).shape)


# revision 23
# speedup vs baseline: 1.1610x; 1.1610x over previous
"""Trainium2 Bass kernel for nn_EssentialMatixModule.

Dual-softmax cross-attention (LoFTR-style) + bilinear feature + projection.
Data-parallel over batch across 8 cores; proj output-sharded with chunked
AllGathers of the (bf16) feature matrix overlapping the attention phase.

v2: x pre-scaled by 1/sigma (column broadcast via DMA) so QKV psum
evacuations are pure casts on the scalar engine; E^2 on DVE 2x mode; zc via
DVE pair-sums + 6 matmuls; 1/zr folded onto vpl, 1/zc onto vpc (both 4x
tensor_scalar); single-bank up psum with one batched evacuation; 3-deep
S-psum rotation and interleaved PE emission for a dense matmul stream.
"""

import sys

sys.path.insert(0, "/opt/trn_rl_repo")

from contextlib import ExitStack

import ml_dtypes
import numpy as np

import concourse.bass as bass
import concourse.tile as tile
from concourse import bacc, mybir
from concourse.bass_utils import run_bass_kernel_spmd

B, C, HG, WG = 64, 256, 24, 24
N = HG * WG  # 576
H, HD = 3, 64
F = H * HD  # 192
SCALE = HD**-0.5
EPS = 1e-5
NCORES = 8
BP = B // NCORES  # 8 items per core
NT = [128, 128, 128, 128, 64]  # token tiles (sum=576)
NCH = [(0, 512), (512, 64)]  # free-dim chunks for N=576 psum
DE = 70  # hd + 6 pos dims
PADMH = 4992  # 39*128, per-(map,head) padded feat block
DIMS = 6 * PADMH  # 29952
OS = 512 // NCORES  # 64 output cols per core
F32 = mybir.dt.float32
BF16 = mybir.dt.bfloat16
AX = mybir.ActivationFunctionType
OP = mybir.AluOpType


def _host_prep(ln_w, ln_b, qkv_w, proj_w, proj_b):
    ln_w = ln_w.astype(np.float64)
    ln_b = ln_b.astype(np.float64)
    qw = qkv_w.astype(np.float64)
    Wp = qw * ln_w[None, :]  # [576, C]
    r = Wp.sum(axis=1)  # [576]
    t = qw @ ln_b  # [576]

    # per-side packing: side0 tiles hold [k_h; q_h], side1 [q_h; k_h] so the
    # attention matmul operands always share a partition base
    def col(fsl, scale):
        return np.concatenate([Wp[fsl] * scale, (r[fsl] * scale)[:, None],
                               (t[fsl] * scale)[:, None]], axis=1).T

    wqk = np.zeros((2, C + 2, 3 * 128), np.float32)
    for h in range(H):
        qr = slice(h * HD, (h + 1) * HD)
        kr = slice(F + h * HD, F + (h + 1) * HD)
        qcols = col(qr, SCALE)  # [C+2, 64]
        kcols = col(kr, 1.0)
        wqk[0, :, h * 128 : h * 128 + 64] = kcols
        wqk[0, :, h * 128 + 64 : h * 128 + 128] = qcols
        wqk[1, :, h * 128 : h * 128 + 64] = qcols
        wqk[1, :, h * 128 + 64 : h * 128 + 128] = kcols
    wqk = wqk.astype(ml_dtypes.bfloat16)

    wv = np.zeros((C + 2, F), np.float32)
    wv[:C] = Wp[2 * F :].T
    wv[C] = r[2 * F :]
    wv[C + 1] = t[2 * F :]
    wv = wv.astype(ml_dtypes.bfloat16)

    ys = np.linspace(-1.0, 1.0, HG)
    xs = np.linspace(-1.0, 1.0, WG)
    p3 = np.tile(ys, WG)
    p4 = np.repeat(xs, HG)
    pos = np.stack([p3 * p3, p4 * p4, p3 * p4, p3, p4, np.ones_like(p3)], axis=1)
    pos_pad = np.zeros((640, 6), np.float32)
    pos_pad[:N] = pos

    pwt = np.zeros((DIMS, 512), np.float32)
    for mh in range(6):
        blk = proj_w[:, mh * 4900 : (mh + 1) * 4900]  # [512, 4900]
        pwt[mh * PADMH : mh * PADMH + 4900] = blk.T
    pwt = pwt.astype(ml_dtypes.bfloat16)
    return wqk, wv, pos_pad, pwt


def _build():
    nc = bacc.Bacc()
    x1d = nc.declare_dram_parameter("x1s", [BP, C, N], BF16, isOutput=False)
    x2d = nc.declare_dram_parameter("x2s", [BP, C, N], BF16, isOutput=False)
    wqkd = nc.declare_dram_parameter("wqk", [2, C + 2, 3 * 128], BF16, isOutput=False)
    wvd = nc.declare_dram_parameter("wv", [C + 2, F], BF16, isOutput=False)
    posd = nc.declare_dram_parameter("pos", [640, 6], F32, isOutput=False)
    pwtd = nc.declare_dram_parameter("pwt", [DIMS, OS], BF16, isOutput=False)
    pbd = nc.declare_dram_parameter("pb", [1, OS], F32, isOutput=False)
    outd = nc.declare_dram_parameter("out", [B, OS], F32, isOutput=True)
    # per side: row0 = -mu/sigma, row1 = 1/sigma  (bf16), [2, 2, BP, N]
    statsd = nc.dram_tensor("statsd", [2, 2, BP, N], BF16)
    statsdf = nc.dram_tensor("statsdf", [2, BP, N], F32)
    feat8d = [nc.dram_tensor(f"feat8_{j}", [BP, PADMH], BF16) for j in range(6)]
    featAG = [
        nc.dram_tensor(f"featAG_{j}", [B, PADMH], BF16, addr_space="Shared")
        for j in range(6)
    ]
    xd = [x1d, x2d]

    def bcast_p(sl, p):
        return bass.AP(tensor=sl.tensor, offset=sl.offset, ap=[[0, p]] + list(sl.ap))

    with ExitStack() as ctx:
        tc = ctx.enter_context(tile.TileContext(nc))
        const = ctx.enter_context(tc.tile_pool(name="const", bufs=1))
        xin = ctx.enter_context(tc.tile_pool(name="xin", bufs=6))
        stats = ctx.enter_context(tc.tile_pool(name="stats", bufs=1))
        tmp = ctx.enter_context(tc.tile_pool(name="tmp", bufs=4))
        isbp = ctx.enter_context(tc.tile_pool(name="isbp", bufs=3))
        sb_qk = ctx.enter_context(tc.tile_pool(name="sbqk", bufs=1))
        sb_vp = ctx.enter_context(tc.tile_pool(name="sbvp", bufs=1))
        epool = ctx.enter_context(tc.tile_pool(name="epool", bufs=12))
        e2pool = ctx.enter_context(tc.tile_pool(name="e2pool", bufs=12))
        espool = ctx.enter_context(tc.tile_pool(name="espool", bufs=3))
        zpool = ctx.enter_context(tc.tile_pool(name="zpool", bufs=6))
        upool = ctx.enter_context(tc.tile_pool(name="upool", bufs=12))
        uspool = ctx.enter_context(tc.tile_pool(name="uspool", bufs=4))
        fpool = ctx.enter_context(tc.tile_pool(name="fpool", bufs=4))
        ftpool = ctx.enter_context(tc.tile_pool(name="ftpool", bufs=3))
        opool = ctx.enter_context(tc.tile_pool(name="opool", bufs=2))
        # PSUM: S-pool 3x2 banks, up 1 bank, fps(+proj) 1 bank
        psS = ctx.enter_context(tc.tile_pool(name="psS", bufs=3, space="PSUM"))
        psU = ctx.enter_context(tc.tile_pool(name="psU", bufs=1, space="PSUM"))
        psF = ctx.enter_context(tc.tile_pool(name="psF", bufs=1, space="PSUM"))

        # ---- constants ----
        wqk_sb = [[], []]
        wv_sb = []
        for k, (k0, kw) in enumerate([(0, 128), (128, 128), (256, 2)]):
            for s in range(2):
                wt = const.tile([kw, 3 * 128], BF16, tag=f"wqk{s}_{k}")
                nc.sync.dma_start(out=wt, in_=wqkd[s, k0 : k0 + kw, :])
                wqk_sb[s].append(wt)
            vt = const.tile([kw, F], BF16, tag=f"wv{k}")
            nc.sync.dma_start(out=vt, in_=wvd[k0 : k0 + kw, :])
            wv_sb.append(vt)
        pos_sb = const.tile([128, 5, 6], F32, tag="pos")
        nc.sync.dma_start(out=pos_sb, in_=posd.rearrange("(t p) e -> p t e", p=128))
        ind8_sb = const.tile([128, BP, BP], BF16, tag="ind8")
        nc.vector.memset(ind8_sb, 0.0)
        for i in range(BP):
            nc.vector.memset(ind8_sb[:, i, i : i + 1], 1.0)
        onesb_sb = const.tile([128, 32], BF16, tag="onesb")
        nc.vector.memset(onesb_sb, 1.0)
        epssb = const.tile([32, 1], F32, tag="eps")
        nc.vector.memset(epssb, EPS)
        pb_sb = const.tile([B, OS], F32, tag="pb")
        nc.gpsimd.dma_start(out=pb_sb, in_=bcast_p(pbd[0, :], B))
        zpad = const.tile([1, 552], BF16, tag="zpad")
        nc.vector.memset(zpad, 0.0)
        for j in range(6):
            for ib in range(BP):
                nc.gpsimd.dma_start(
                    out=feat8d[j][ib, 4900:PADMH],
                    in_=bass.AP(tensor=zpad.tensor, offset=zpad.offset, ap=[[1, 1], [1, 92]]),
                )
        # negmu_flat[s]: [2, BP*N] row0 = -mu/sigma (dma later), row1 = ones
        nmf = []
        for s in range(2):
            t_ = const.tile([2, BP, N], BF16, tag=f"nmf{s}")
            nc.vector.memset(t_, 1.0)  # row 0 overwritten by -mu/sigma DMA later
            nmf.append(t_)

        # ---- phase 1a: LN stats (batched per side) ----
        for s in range(2):
            psum_s = psS.tile([128, N], F32, tag="pq", name="psum_s")[0:BP]
            psum_q = psS.tile([128, N], F32, tag="pq", name="psum_q")[0:BP]
            for i in range(BP):
                for k in range(2):
                    xt = xin.tile([128, N], BF16, tag="x")
                    nc.scalar.dma_start(out=xt, in_=xd[s][i, k * 128 : (k + 1) * 128, :])
                    xq = tmp.tile([128, N], BF16, tag="xsq")
                    nc.vector.tensor_mul(xq, xt, xt)
                    st = i == 0 and k == 0
                    for c0, cw in NCH:
                        nc.tensor.matmul(
                            psum_s[:, c0 : c0 + cw], ind8_sb[:, i, :], xt[:, c0 : c0 + cw],
                            start=st, stop=(i == BP - 1 and k == 1),
                        )
                        nc.tensor.matmul(
                            psum_q[:, c0 : c0 + cw], ind8_sb[:, i, :], xq[:, c0 : c0 + cw],
                            start=st, stop=(i == BP - 1 and k == 1),
                        )
            mean = stats.tile([32, N], F32, tag="mean")
            ex2 = stats.tile([32, N], F32, tag="ex2")
            nc.vector.tensor_scalar_mul(mean[:BP], psum_s, 1.0 / C)
            nc.vector.tensor_scalar_mul(ex2[:BP], psum_q, 1.0 / C)
            var = stats.tile([32, N], F32, tag="var")
            nc.vector.scalar_tensor_tensor(
                out=var[:BP], in0=mean[:BP], scalar=-1.0, in1=mean[:BP], op0=OP.mult, op1=OP.mult
            )
            nc.vector.tensor_add(var[:BP], var[:BP], ex2[:BP])
            sig = stats.tile([32, N], F32, tag="sig")
            nc.scalar.activation(out=sig[:BP], in_=var[:BP], func=AX.Sqrt, bias=epssb[:BP])
            isv = stats.tile([32, N], F32, tag="isv")
            nc.vector.reciprocal(isv[:BP], sig[:BP])
            # -mu/sigma and 1/sigma, bf16
            nmu_is = stats.tile([32, N], BF16, tag="nmu_is")
            nc.vector.scalar_tensor_tensor(
                out=nmu_is[:BP], in0=mean[:BP], scalar=-1.0, in1=isv[:BP],
                op0=OP.mult, op1=OP.mult,
            )
            nc.sync.dma_start(out=statsd[s, 0], in_=nmu_is[:BP])
            nc.sync.dma_start(out=statsdf[s], in_=isv[:BP])
            # flat [-mu/sigma] row at partition 0 (for k=2 matmul operands)
            nc.sync.dma_start(
                out=nmf[s][0:1, :, :].rearrange("p b n -> p (b n)"),
                in_=bcast_p(statsd[s, 0].rearrange("b n -> (b n)"), 1),
            )

        # ---- phase 1b: QKV for all items (x pre-scaled by 1/sigma) ----
        qs = {}
        ks = {}
        vp = {}
        for i in range(BP):
            for s in range(2):
                # broadcast 1/sigma row of item i across 128 partitions via DMA
                isb = isbp.tile([128, N], F32, tag="isb")
                nc.gpsimd.dma_start(out=isb, in_=bcast_p(statsdf[s, i, :], 128))
                xt0 = xin.tile([128, N], BF16, tag="x")
                nc.sync.dma_start(out=xt0, in_=xd[s][i, 0:128, :])
                xt1 = xin.tile([128, N], BF16, tag="x")
                nc.sync.dma_start(out=xt1, in_=xd[s][i, 128:256, :])
                nc.vector.tensor_mul(xt0, xt0, isb)
                nc.vector.tensor_mul(xt1, xt1, isb)
                rhs3 = [xt0, xt1, nmf[s][:, i, :]]
                # per-side packed [k;q] (s=0) / [q;k] (s=1) tiles
                for h in range(H):
                    pq = psS.tile([128, N], F32, tag="pq")
                    for k in range(3):
                        for c0, cw in NCH:
                            nc.tensor.matmul(
                                pq[:, c0 : c0 + cw],
                                wqk_sb[s][k][:, h * 128 : (h + 1) * 128],
                                rhs3[k][:, c0 : c0 + cw],
                                start=(k == 0), stop=(k == 2),
                            )
                    qk = sb_qk.tile([128, N], BF16, tag=f"qk{i}_{s}_{h}")
                    nc.scalar.activation(out=qk, in_=pq, func=AX.Copy)
                    if s == 0:
                        ks[i, s, h] = qk[0:64, :]
                        qs[i, s, h] = qk[64:128, :]
                    else:
                        qs[i, s, h] = qk[0:64, :]
                        ks[i, s, h] = qk[64:128, :]
                for nt in range(5):
                    w = NT[nt]
                    n0 = nt * 128
                    pv = psS.tile([128, N], F32, tag="pq")
                    for k in range(2):
                        nc.tensor.matmul(
                            pv[:w, 0:F],
                            rhs3[k][:, n0 : n0 + w],
                            wv_sb[k],
                            start=(k == 0), stop=False,
                        )
                    nc.tensor.matmul(
                        pv[:w, 0:F],
                        nmf[s][:, i, n0 : n0 + w],
                        wv_sb[2],
                        start=False, stop=True,
                    )
                    vt = sb_vp.tile([128, 3, 72], BF16, tag=f"vp{i}_{s}_{nt}")
                    nc.vector.tensor_copy(
                        out=vt[:w, :, 0:64],
                        in_=pv[:w, 0:F].rearrange("p (a b) -> p a b", b=64),
                    )
                    ps = pos_sb[:w, nt, :]
                    nc.vector.tensor_copy(
                        out=vt[:w, :, 64:70],
                        in_=bass.AP(tensor=ps.tensor, offset=ps.offset,
                                    ap=[ps.ap[0], [0, 3], ps.ap[-1]]),
                    )
                    vp[i, s, nt] = vt

        # ---- phase 2: attention, software-pipelined; AllGather + proj ----
        oacc = opool.tile([B, OS], F32, tag="oacc")
        nc.vector.memset(oacc, 0.0)

        def emit_proj(mh):
            GSZ = 13
            for g0 in range(0, 39, GSZ):
                ft = ftpool.tile([128, GSZ, B], BF16, tag="ft")
                nc.sync.dma_start_transpose(
                    out=ft, in_=featAG[mh][:, g0 * 128 : (g0 + GSZ) * 128]
                )
                pw = ftpool.tile([128, GSZ, OS], BF16, tag="pw")
                nc.gpsimd.dma_start(
                    out=pw,
                    in_=pwtd[mh * PADMH + g0 * 128 : mh * PADMH + (g0 + GSZ) * 128, :]
                    .rearrange("(j p) o -> p j o", p=128),
                )
                opsum = psF.tile([64, OS], F32, tag="fps")
                for j in range(GSZ):
                    nc.tensor.matmul(
                        opsum, ft[:, j, :], pw[:, j, :],
                        start=(j == 0), stop=(j == GSZ - 1),
                    )
                nc.vector.tensor_add(oacc, oacc, opsum)

        def emit_gather(j):
            nc.gpsimd.collective_compute(
                "AllGather",
                OP.bypass,
                replica_groups=[list(range(NCORES))],
                ins=[feat8d[j][:]],
                outs=[featAG[j][:]],
            )

        # block order: (h, i) pairs with m inner so adjacent blocks use
        # disjoint PE row halves (m=0: rows 0-63, m=1: rows 64-127)
        blocks = []
        for h in range(H):
            for i in range(BP):
                for m in range(2):
                    blocks.append((m, h, i))
        NB = len(blocks)
        NP = NB // 2

        # per-block live state
        S_ps = {}     # (bi, nt) -> S psum tile
        E_t = {}      # (bi, nt) -> E sbuf bf16
        E2_t = {}     # (bi, nt) -> E^2 sbuf bf16
        ES_t = {}     # (bi, pair) -> E pair sums
        zr5 = {}      # bi -> [128, 8] f32 accum of exp row sums
        rzr5 = {}     # bi -> reciprocal
        zc_ps = {}    # bi -> zc psum [32, N]
        vpl_t = {}    # (bi, nt) -> vp scaled by 1/zr
        vpc_t = {}    # (bi, mc) -> vp scaled by 1/zc
        rzc_t = {}    # bi -> [128, 5] per-partition 1/zc
        up_ps = {}    # bi -> up psum [128, 5, 72]
        us_t = {}     # bi -> up evac sbuf
        fps_ps = {}   # bi -> fps psum [70, 72]

        proj_at = {3: [0, 1], 5: [2, 3]}
        gather_at = {1: [0, 1], 3: [2, 3], 4: [4]}

        def qk_mm(bi, nt):
            m, h, i = blocks[bi]
            qside, vside = 1 - m, m
            w = NT[nt]
            n0 = nt * 128
            pa = psS.tile([128, N], F32, tag="pq", name="pa")
            S_ps[bi, nt] = pa
            for c0, cw in NCH:
                nc.tensor.matmul(
                    pa[:w, c0 : c0 + cw],
                    qs[i, qside, h][:, n0 : n0 + w],
                    ks[i, vside, h][:, c0 : c0 + cw],
                    start=True, stop=True,
                )

        def qk_pair(ba, bb, nt):
            qk_mm(ba, nt)
            qk_mm(bb, nt)

        def exp_act(bi, nt):
            w = NT[nt]
            if nt == 0:
                zr5[bi] = zpool.tile([128, 8], F32, tag="zr5", name="zr5")
            et = epool.tile([128, N], BF16, tag="E", name="et")
            E_t[bi, nt] = et
            nc.scalar.activation(
                out=et[:w], in_=S_ps.pop((bi, nt))[:w], func=AX.Exp,
                accum_out=zr5[bi][:w, nt : nt + 1],
            )

        def e2_mul(bi, nt):
            w = NT[nt]
            e2 = e2pool.tile([128, N], BF16, tag="E2", name="e2")
            E2_t[bi, nt] = e2
            nc.vector.tensor_mul(e2[:w], E_t[bi, nt][:w], E_t[bi, nt][:w])

        def esum(bi):
            # full tree-reduce of the 5 E tiles to one [128, N] tile so the
            # zc colsum matmul streams 576 rows instead of 1728
            es0 = espool.tile([128, N], BF16, tag="es0", name="es0", bufs=2)
            nc.vector.tensor_add(es0, E_t[bi, 0], E_t[bi, 1])
            es1 = espool.tile([128, N], BF16, tag="es1", name="es1", bufs=2)
            nc.gpsimd.tensor_add(es1, E_t[bi, 2], E_t[bi, 3])
            es = espool.tile([128, N], BF16, tag="es2", name="es2")
            nc.vector.tensor_add(es, es0, es1)
            nc.vector.tensor_add(es[0:64], es[0:64], E_t[bi, 4][0:64])
            ES_t[bi] = es

        def rzr(bi):
            r = zpool.tile([128, 8], F32, tag="rzr5", name="rzr5")
            rzr5[bi] = r
            nc.vector.reciprocal(r[:, 0:5], zr5[bi][:, 0:5])

        def vpl_scale(bi, nt):
            m, h, i = blocks[bi]
            vside = m
            w = NT[nt]
            vpl = upool.tile([128, 72], BF16, tag="vpl", name="vpl")
            vpl_t[bi, nt] = vpl
            nc.vector.tensor_scalar_mul(
                vpl[:w, 0:70], vp[i, vside, nt][:w, h, 0:70],
                rzr5[bi][:w, nt : nt + 1],
            )

        def zc_mms(bi):
            zcp = psS.tile([128, N], F32, tag="pq", name="zcp")[0:32]
            zc_ps[bi] = zcp
            src_ = ES_t[bi]
            for c0, cw in NCH:
                nc.tensor.matmul(
                    zcp[:, c0 : c0 + cw], onesb_sb[:, :], src_[:, c0 : c0 + cw],
                    start=True, stop=True,
                )

        def rzc(bi):
            zt = tmp.tile([32, 18, 32], F32, tag="zt")
            nc.vector.transpose(
                out=zt, in_=zc_ps.pop(bi).rearrange("p (g q) -> p g q", q=32)
            )
            rz32 = zpool.tile([32, 18], F32, tag="rz32")
            nc.vector.reciprocal(rz32, zt[:, :, 0])
            r = zpool.tile([128, 5], F32, tag="rzc", name="rzc")
            rzc_t[bi] = r
            for a in range(4):
                ng = 5 if a < 2 else 4
                nc.vector.tensor_copy(
                    out=r[32 * a : 32 * a + 32, 0:ng],
                    in_=rz32[:, a : 18 : 4],
                )

        def vpc_scale(bi, mc):
            m, h, i = blocks[bi]
            vside = m
            w2 = NT[mc]
            vpc = upool.tile([128, 72], BF16, tag="vpc", name="vpc")
            vpc_t[bi, mc] = vpc
            nc.vector.tensor_scalar_mul(
                vpc[:w2, 0:70], vp[i, vside, mc][:w2, h, 0:70],
                rzc_t[bi][:w2, mc : mc + 1],
            )

        def up_mms(bi, nt):
            w = NT[nt]
            if nt == 0:
                up_ps[bi] = psU.tile([128, 5, 72], F32, tag="up", name="up")
            upt = up_ps[bi]
            e2 = E2_t[bi, nt]
            vpl = vpl_t[bi, nt]
            # single accumulation group for the whole bank: start=True clears
            # has_written for the entire 2KB bank, so per-mc groups would wipe
            # each other's partials. One group + per-element has_written works.
            for mc in range(5):
                w2 = NT[mc]
                nc.tensor.matmul(
                    upt[:w2, mc, 0:70],
                    e2[:w, mc * 128 : mc * 128 + w2],
                    vpl[:w, 0:70],
                    start=(nt == 0 and mc == 0), stop=(nt == 4 and mc == 4),
                    skip_group_check=True,
                )

        def up_evac(bi, on_act):
            us = uspool.tile([128, 5, 72], BF16, tag="us", name="us")
            us_t[bi] = us
            upt = up_ps.pop(bi)
            # read only matmul-written psum regions (mc 0-3 full, mc 4 rows 0-63)
            if on_act:
                nc.scalar.activation(out=us[:, 0:4, 0:70], in_=upt[:, 0:4, 0:70], func=AX.Copy)
                nc.scalar.activation(out=us[0:64, 4, 0:70], in_=upt[0:64, 4, 0:70], func=AX.Copy)
            else:
                nc.vector.tensor_copy(out=us[:, 0:4, 0:70], in_=upt[:, 0:4, 0:70])
                nc.vector.tensor_copy(out=us[0:64, 4, 0:70], in_=upt[0:64, 4, 0:70])
            # drop E2/vpl refs
            for nt in range(5):
                E2_t.pop((bi, nt))
                vpl_t.pop((bi, nt))

        def fps_mms(bi):
            fp = psF.tile([70, 72], F32, tag="fps", name="fps")
            fps_ps[bi] = fp
            us = us_t.pop(bi)
            for mc in range(5):
                w2 = NT[mc]
                nc.tensor.matmul(
                    fp[0:70, 0:70],
                    us[:w2, mc, 0:70],
                    vpc_t.pop((bi, mc))[:w2, 0:70],
                    start=(mc == 0), stop=(mc == 4),
                )

        def fb_out(bi):
            m, h, i = blocks[bi]
            mh = m * 3 + h
            fb = fpool.tile([70, 70], BF16, tag="fb", name="fb")
            nc.vector.tensor_copy(out=fb, in_=fps_ps.pop(bi)[0:70, 0:70])
            nc.sync.dma_start(
                out=feat8d[mh][i, 0:4900].rearrange("(d e) -> d e", e=70),
                in_=fb,
            )

        def drop_e(bi):
            for nt in range(5):
                E_t.pop((bi, nt))
            ES_t.pop(bi)
            zr5.pop(bi)
            rzr5.pop(bi)
            rzc_t.pop(bi)

        # prologue: QK + exp for pair 0 (blocks 0, 1), interleaved A/B
        for nt in range(5):
            qk_pair(0, 1, nt)
        for nt in range(5):
            exp_act(0, nt)
            exp_act(1, nt)

        # gather j after the fb of its (h, i=7) pair lands (pair-iter 8h+7+2);
        # proj j a few pair-iters after its gather
        gather_sched = {10: [0, 3], 18: [1, 4]}
        proj_sched = {13: [0], 16: [3], 21: [1], 24: [4]}

        # steady state over pairs: iteration p emits QK(p), math(p-1), out(p-2)
        evac_q = []
        for p in range(1, NP + 3):
            A, Bb = 2 * p, 2 * p + 1          # current pair blocks
            A1, B1 = A - 2, Bb - 2            # math stage
            A2, B2 = A - 6, Bb - 6            # out stage (after up_evac lag)
            for pj in proj_sched.get(p, []):
                emit_proj(pj)
            # DVE: evacuate previous pair's up psums first (frees psU bank),
            # then the math-front for (A1, B1)
            for X in evac_q:
                up_evac(X, on_act=False)
                for mc in range(5):
                    vpc_scale(X, mc)
            evac_q = []
            if A1 < NB:
                for X in (A1, B1):
                    rzr(X)
                    for nt in range(5):
                        e2_mul(X, nt)
                        vpl_scale(X, nt)
                    esum(X)
            # PE: QK pairs interleaved with fps of the out stage
            if p < NP:
                qk_pair(A, Bb, 0)
            if 0 <= A2 < NB:
                fps_mms(A2)
            if p < NP:
                qk_pair(A, Bb, 1)
            if 0 <= B2 < NB:
                fps_mms(B2)
            if p < NP:
                for nt in range(2, 5):
                    qk_pair(A, Bb, nt)
            if A1 < NB:
                for nt in range(5):
                    up_mms(A1, nt)
                zc_mms(A1)
                for nt in range(5):
                    up_mms(B1, nt)
                zc_mms(B1)
                evac_q = [A1, B1]
                    # ACT: exps for pair p in S-pool allocation order
            if p < NP:
                for nt in range(5):
                    exp_act(A, nt)
                    exp_act(Bb, nt)
            if A1 < NB:
                rzc(A1)
                rzc(B1)
            if 0 <= A2 < NB:
                fb_out(A2)
                fb_out(B2)
                drop_e(A2)
                drop_e(B2)
            for j in gather_sched.get(p, []):
                emit_gather(j)

        emit_gather(2)
        emit_gather(5)
        emit_proj(2)
        emit_proj(5)
        osb = opool.tile([B, OS], F32, tag="osb")
        nc.vector.tensor_add(osb, oacc, pb_sb)
        nc.vector.tensor_scalar_max(osb, osb, 0.0)
        nc.sync.dma_start(out=outd[:], in_=osb)

    nc.compile()
    return nc


def kernel(x1, x2, ln_w, ln_b, qkv_w, proj_w, proj_b):
    wqk, wv, pos_pad, pwt = _host_prep(ln_w, ln_b, qkv_w, proj_w, proj_b)
    xs1 = np.ascontiguousarray(x1.reshape(B, C, N)).astype(ml_dtypes.bfloat16)
    xs2 = np.ascontiguousarray(x2.reshape(B, C, N)).astype(ml_dtypes.bfloat16)
    nc = _build()
    in_maps = []
    for r in range(NCORES):
        in_maps.append(
            {
                "x1s": xs1[r * BP : (r + 1) * BP],
                "x2s": xs2[r * BP : (r + 1) * BP],
                "wqk": wqk,
                "wv": wv,
                "pos": pos_pad,
                "pwt": np.ascontiguousarray(pwt[:, r * OS : (r + 1) * OS]),
                "pb": np.ascontiguousarray(proj_b[None, r * OS : (r + 1) * OS]).astype(np.float32),
            }
        )
    import os

    trace = bool(os.environ.get("BASS_TRACE"))
    res = run_bass_kernel_spmd(nc, in_maps, core_ids=list(range(NCORES)), trace=trace)
    if res.exec_time_ns is not None:
        print(f"HW exec time: {res.exec_time_ns} ns")
    if res.instructions_and_trace:
        print("trace path:", res.instructions_and_trace[1])
    out = np.concatenate([res.results[r]["out"] for r in range(NCORES)], axis=1)
    return out.astype(np.float32)


if __name__ == "__main__":
    rng = np.random.default_rng(0)
    ins = {
        "x1": rng.standard_normal((B, C, HG, WG), dtype=np.float32),
        "x2": rng.standard_normal((B, C, HG, WG), dtype=np.float32),
        "ln_w": np.ones(C, np.float32),
        "ln_b": np.zeros(C, np.float32),
        "qkv_w": (rng.standard_normal((3 * F, C)) * C**-0.5).astype(np.float32),
        "proj_w": (rng.standard_normal((512, 6 * 4900)) * (6 * 4900) ** -0.5).astype(np.float32),
        "proj_b": np.zeros(512, np.float32),
    }
    print(kernel(**ins).shape)


# revision 26
# speedup vs baseline: 1.1656x; 1.0040x over previous
"""Trainium2 Bass kernel for nn_EssentialMatixModule.

Dual-softmax cross-attention (LoFTR-style) + bilinear feature + projection.
Data-parallel over batch across 8 cores; proj output-sharded with chunked
AllGathers of the (bf16) feature matrix overlapping the attention phase.

v2: x pre-scaled by 1/sigma (column broadcast via DMA) so QKV psum
evacuations are pure casts on the scalar engine; E^2 on DVE 2x mode; zc via
DVE pair-sums + 6 matmuls; 1/zr folded onto vpl, 1/zc onto vpc (both 4x
tensor_scalar); single-bank up psum with one batched evacuation; 3-deep
S-psum rotation and interleaved PE emission for a dense matmul stream.
"""

import sys

sys.path.insert(0, "/opt/trn_rl_repo")

from contextlib import ExitStack

import ml_dtypes
import numpy as np

import concourse.bass as bass
import concourse.tile as tile
from concourse import bacc, mybir
from concourse.bass_utils import run_bass_kernel_spmd

B, C, HG, WG = 64, 256, 24, 24
N = HG * WG  # 576
H, HD = 3, 64
F = H * HD  # 192
SCALE = HD**-0.5
EPS = 1e-5
NCORES = 8
BP = B // NCORES  # 8 items per core
NT = [128, 128, 128, 128, 64]  # token tiles (sum=576)
NCH = [(0, 512), (512, 64)]  # free-dim chunks for N=576 psum
DE = 70  # hd + 6 pos dims
PADMH = 4992  # 39*128, per-(map,head) padded feat block
DIMS = 6 * PADMH  # 29952
OS = 512 // NCORES  # 64 output cols per core
F32 = mybir.dt.float32
BF16 = mybir.dt.bfloat16
AX = mybir.ActivationFunctionType
OP = mybir.AluOpType


def _host_prep(ln_w, ln_b, qkv_w, proj_w, proj_b):
    ln_w = ln_w.astype(np.float64)
    ln_b = ln_b.astype(np.float64)
    qw = qkv_w.astype(np.float64)
    Wp = qw * ln_w[None, :]  # [576, C]
    r = Wp.sum(axis=1)  # [576]
    t = qw @ ln_b  # [576]

    # per-side packing: side0 tiles hold [k_h; q_h], side1 [q_h; k_h] so the
    # attention matmul operands always share a partition base
    def col(fsl, scale):
        return np.concatenate([Wp[fsl] * scale, (r[fsl] * scale)[:, None],
                               (t[fsl] * scale)[:, None]], axis=1).T

    wqk = np.zeros((2, C + 2, 3 * 128), np.float32)
    for h in range(H):
        qr = slice(h * HD, (h + 1) * HD)
        kr = slice(F + h * HD, F + (h + 1) * HD)
        qcols = col(qr, SCALE)  # [C+2, 64]
        kcols = col(kr, 1.0)
        wqk[0, :, h * 128 : h * 128 + 64] = kcols
        wqk[0, :, h * 128 + 64 : h * 128 + 128] = qcols
        wqk[1, :, h * 128 : h * 128 + 64] = qcols
        wqk[1, :, h * 128 + 64 : h * 128 + 128] = kcols
    wqk = wqk.astype(ml_dtypes.bfloat16)

    wv = np.zeros((C + 2, F), np.float32)
    wv[:C] = Wp[2 * F :].T
    wv[C] = r[2 * F :]
    wv[C + 1] = t[2 * F :]
    wv = wv.astype(ml_dtypes.bfloat16)

    ys = np.linspace(-1.0, 1.0, HG)
    xs = np.linspace(-1.0, 1.0, WG)
    p3 = np.tile(ys, WG)
    p4 = np.repeat(xs, HG)
    pos = np.stack([p3 * p3, p4 * p4, p3 * p4, p3, p4, np.ones_like(p3)], axis=1)
    pos_pad = np.zeros((640, 6), np.float32)
    pos_pad[:N] = pos

    pwt = np.zeros((DIMS, 512), np.float32)
    for mh in range(6):
        blk = proj_w[:, mh * 4900 : (mh + 1) * 4900]  # [512, 4900]
        pwt[mh * PADMH : mh * PADMH + 4900] = blk.T
    pwt = pwt.astype(ml_dtypes.bfloat16)
    # per-feature bias terms (row C+1 of the packed weights), folded into
    # the psum evacuations instead of rank-2 bias matmuls
    tqkT = np.ascontiguousarray(
        wqk[:, C + 1, :].astype(np.float32).reshape(2, 3, 128).transpose(0, 2, 1)
    )  # [2, 128, 3]
    tv = wv[C + 1 : C + 2, :].astype(np.float32)  # [1, F]
    return wqk, wv, pos_pad, pwt, tqkT, tv


def _build():
    nc = bacc.Bacc()
    x1d = nc.declare_dram_parameter("x1s", [BP, C, N], BF16, isOutput=False)
    x2d = nc.declare_dram_parameter("x2s", [BP, C, N], BF16, isOutput=False)
    wqkd = nc.declare_dram_parameter("wqk", [2, C + 2, 3 * 128], BF16, isOutput=False)
    wvd = nc.declare_dram_parameter("wv", [C + 2, F], BF16, isOutput=False)
    posd = nc.declare_dram_parameter("pos", [640, 6], F32, isOutput=False)
    pwtd = nc.declare_dram_parameter("pwt", [DIMS, OS], BF16, isOutput=False)
    pbd = nc.declare_dram_parameter("pb", [1, OS], F32, isOutput=False)
    outd = nc.declare_dram_parameter("out", [B, OS], F32, isOutput=True)
    # per side: row0 = -mu/sigma, row1 = 1/sigma  (bf16), [2, 2, BP, N]
    statsd = nc.dram_tensor("statsd", [2, 2, BP, N], BF16)
    statsdf = nc.dram_tensor("statsdf", [2, BP, N], F32)
    feat8d = [nc.dram_tensor(f"feat8_{j}", [BP, PADMH], BF16) for j in range(6)]
    featAG = [
        nc.dram_tensor(f"featAG_{j}", [B, PADMH], BF16, addr_space="Shared")
        for j in range(6)
    ]
    xd = [x1d, x2d]

    def bcast_p(sl, p):
        return bass.AP(tensor=sl.tensor, offset=sl.offset, ap=[[0, p]] + list(sl.ap))

    with ExitStack() as ctx:
        tc = ctx.enter_context(tile.TileContext(nc))
        const = ctx.enter_context(tc.tile_pool(name="const", bufs=1))
        xin = ctx.enter_context(tc.tile_pool(name="xin", bufs=6))
        stats = ctx.enter_context(tc.tile_pool(name="stats", bufs=1))
        tmp = ctx.enter_context(tc.tile_pool(name="tmp", bufs=4))
        isbp = ctx.enter_context(tc.tile_pool(name="isbp", bufs=3))
        sb_qk = ctx.enter_context(tc.tile_pool(name="sbqk", bufs=1))
        sb_vp = ctx.enter_context(tc.tile_pool(name="sbvp", bufs=1))
        epool = ctx.enter_context(tc.tile_pool(name="epool", bufs=12))
        e2pool = ctx.enter_context(tc.tile_pool(name="e2pool", bufs=12))
        espool = ctx.enter_context(tc.tile_pool(name="espool", bufs=3))
        zpool = ctx.enter_context(tc.tile_pool(name="zpool", bufs=6))
        upool = ctx.enter_context(tc.tile_pool(name="upool", bufs=12))
        uspool = ctx.enter_context(tc.tile_pool(name="uspool", bufs=4))
        fpool = ctx.enter_context(tc.tile_pool(name="fpool", bufs=4))
        ftpool = ctx.enter_context(tc.tile_pool(name="ftpool", bufs=3))
        opool = ctx.enter_context(tc.tile_pool(name="opool", bufs=2))
        # PSUM: S-pool 3x2 banks, up 1 bank, fps(+proj) 1 bank
        psS = ctx.enter_context(tc.tile_pool(name="psS", bufs=3, space="PSUM"))
        psU = ctx.enter_context(tc.tile_pool(name="psU", bufs=1, space="PSUM"))
        psF = ctx.enter_context(tc.tile_pool(name="psF", bufs=1, space="PSUM"))

        # ---- constants ----
        wqk_sb = [[], []]
        wv_sb = []
        for k, (k0, kw) in enumerate([(0, 128), (128, 128), (256, 2)]):
            for s in range(2):
                wt = const.tile([kw, 3 * 128], BF16, tag=f"wqk{s}_{k}")
                nc.sync.dma_start(out=wt, in_=wqkd[s, k0 : k0 + kw, :])
                wqk_sb[s].append(wt)
            vt = const.tile([kw, F], BF16, tag=f"wv{k}")
            nc.sync.dma_start(out=vt, in_=wvd[k0 : k0 + kw, :])
            wv_sb.append(vt)
        pos_sb = const.tile([128, 5, 6], F32, tag="pos")
        nc.sync.dma_start(out=pos_sb, in_=posd.rearrange("(t p) e -> p t e", p=128))
        ind8_sb = const.tile([128, BP, BP], BF16, tag="ind8")
        nc.vector.memset(ind8_sb, 0.0)
        for i in range(BP):
            nc.vector.memset(ind8_sb[:, i, i : i + 1], 1.0)
        onesb_sb = const.tile([128, 32], BF16, tag="onesb")
        nc.vector.memset(onesb_sb, 1.0)
        epssb = const.tile([32, 1], F32, tag="eps")
        nc.vector.memset(epssb, EPS)
        pb_sb = const.tile([B, OS], F32, tag="pb")
        nc.gpsimd.dma_start(out=pb_sb, in_=bcast_p(pbd[0, :], B))
        zpad = const.tile([1, 552], BF16, tag="zpad")
        nc.vector.memset(zpad, 0.0)
        for j in range(6):
            for ib in range(BP):
                nc.gpsimd.dma_start(
                    out=feat8d[j][ib, 4900:PADMH],
                    in_=bass.AP(tensor=zpad.tensor, offset=zpad.offset, ap=[[1, 1], [1, 92]]),
                )
        # negmu_flat[s]: [2, BP*N] row0 = -mu/sigma (dma later), row1 = ones
        nmf = []
        for s in range(2):
            t_ = const.tile([2, BP, N], BF16, tag=f"nmf{s}")
            nc.vector.memset(t_, 1.0)  # row 0 overwritten by -mu/sigma DMA later
            nmf.append(t_)

        # ---- phase 1a: LN stats (batched per side) ----
        for s in range(2):
            psum_s = psS.tile([128, N], F32, tag="pq", name="psum_s")[0:BP]
            psum_q = psS.tile([128, N], F32, tag="pq", name="psum_q")[0:BP]
            for i in range(BP):
                for k in range(2):
                    xt = xin.tile([128, N], BF16, tag="x")
                    nc.scalar.dma_start(out=xt, in_=xd[s][i, k * 128 : (k + 1) * 128, :])
                    xq = tmp.tile([128, N], BF16, tag="xsq")
                    nc.vector.tensor_mul(xq, xt, xt)
                    st = i == 0 and k == 0
                    for c0, cw in NCH:
                        nc.tensor.matmul(
                            psum_s[:, c0 : c0 + cw], ind8_sb[:, i, :], xt[:, c0 : c0 + cw],
                            start=st, stop=(i == BP - 1 and k == 1),
                        )
                        nc.tensor.matmul(
                            psum_q[:, c0 : c0 + cw], ind8_sb[:, i, :], xq[:, c0 : c0 + cw],
                            start=st, stop=(i == BP - 1 and k == 1),
                        )
            mean = stats.tile([32, N], F32, tag="mean")
            ex2 = stats.tile([32, N], F32, tag="ex2")
            nc.vector.tensor_scalar_mul(mean[:BP], psum_s, 1.0 / C)
            nc.vector.tensor_scalar_mul(ex2[:BP], psum_q, 1.0 / C)
            var = stats.tile([32, N], F32, tag="var")
            nc.vector.scalar_tensor_tensor(
                out=var[:BP], in0=mean[:BP], scalar=-1.0, in1=mean[:BP], op0=OP.mult, op1=OP.mult
            )
            nc.vector.tensor_add(var[:BP], var[:BP], ex2[:BP])
            sig = stats.tile([32, N], F32, tag="sig")
            nc.scalar.activation(out=sig[:BP], in_=var[:BP], func=AX.Sqrt, bias=epssb[:BP])
            isv = stats.tile([32, N], F32, tag="isv")
            nc.vector.reciprocal(isv[:BP], sig[:BP])
            # -mu/sigma and 1/sigma, bf16
            nmu_is = stats.tile([32, N], BF16, tag="nmu_is")
            nc.vector.scalar_tensor_tensor(
                out=nmu_is[:BP], in0=mean[:BP], scalar=-1.0, in1=isv[:BP],
                op0=OP.mult, op1=OP.mult,
            )
            nc.sync.dma_start(out=statsd[s, 0], in_=nmu_is[:BP])
            nc.sync.dma_start(out=statsdf[s], in_=isv[:BP])
            # flat [-mu/sigma] row at partition 0 (for k=2 matmul operands)
            nc.sync.dma_start(
                out=nmf[s][0:1, :, :].rearrange("p b n -> p (b n)"),
                in_=bcast_p(statsd[s, 0].rearrange("b n -> (b n)"), 1),
            )

        # ---- phase 1b: QKV for all items (x pre-scaled by 1/sigma) ----
        qs = {}
        ks = {}
        vp = {}
        for i in range(BP):
            for s in range(2):
                # broadcast 1/sigma row of item i across 128 partitions via DMA
                isb = isbp.tile([128, N], F32, tag="isb")
                nc.gpsimd.dma_start(out=isb, in_=bcast_p(statsdf[s, i, :], 128))
                xt0 = xin.tile([128, N], BF16, tag="x")
                nc.sync.dma_start(out=xt0, in_=xd[s][i, 0:128, :])
                xt1 = xin.tile([128, N], BF16, tag="x")
                nc.sync.dma_start(out=xt1, in_=xd[s][i, 128:256, :])
                nc.vector.tensor_mul(xt0, xt0, isb)
                nc.vector.tensor_mul(xt1, xt1, isb)
                rhs3 = [xt0, xt1, nmf[s][:, i, :]]
                # per-side packed [k;q] (s=0) / [q;k] (s=1) tiles
                for h in range(H):
                    pq = psS.tile([128, N], F32, tag="pq")
                    for k in range(3):
                        for c0, cw in NCH:
                            nc.tensor.matmul(
                                pq[:, c0 : c0 + cw],
                                wqk_sb[s][k][:, h * 128 : (h + 1) * 128],
                                rhs3[k][:, c0 : c0 + cw],
                                start=(k == 0), stop=(k == 2),
                            )
                    qk = sb_qk.tile([128, N], BF16, tag=f"qk{i}_{s}_{h}")
                    nc.scalar.activation(out=qk, in_=pq, func=AX.Copy)
                    if s == 0:
                        ks[i, s, h] = qk[0:64, :]
                        qs[i, s, h] = qk[64:128, :]
                    else:
                        qs[i, s, h] = qk[0:64, :]
                        ks[i, s, h] = qk[64:128, :]
                for nt in range(5):
                    w = NT[nt]
                    n0 = nt * 128
                    pv = psS.tile([128, N], F32, tag="pq")
                    for k in range(2):
                        nc.tensor.matmul(
                            pv[:w, 0:F],
                            rhs3[k][:, n0 : n0 + w],
                            wv_sb[k],
                            start=(k == 0), stop=False,
                        )
                    nc.tensor.matmul(
                        pv[:w, 0:F],
                        nmf[s][:, i, n0 : n0 + w],
                        wv_sb[2],
                        start=False, stop=True,
                    )
                    vt = sb_vp.tile([128, 3, 72], BF16, tag=f"vp{i}_{s}_{nt}")
                    nc.vector.tensor_copy(
                        out=vt[:w, :, 0:64],
                        in_=pv[:w, 0:F].rearrange("p (a b) -> p a b", b=64),
                    )
                    ps = pos_sb[:w, nt, :]
                    nc.vector.tensor_copy(
                        out=vt[:w, :, 64:70],
                        in_=bass.AP(tensor=ps.tensor, offset=ps.offset,
                                    ap=[ps.ap[0], [0, 3], ps.ap[-1]]),
                    )
                    vp[i, s, nt] = vt

        # ---- phase 2: attention, software-pipelined; AllGather + proj ----
        oacc = opool.tile([B, OS], F32, tag="oacc")
        nc.vector.memset(oacc, 0.0)

        def emit_proj(mh):
            GSZ = 13
            for g0 in range(0, 39, GSZ):
                ft = ftpool.tile([128, GSZ, B], BF16, tag="ft")
                nc.sync.dma_start_transpose(
                    out=ft, in_=featAG[mh][:, g0 * 128 : (g0 + GSZ) * 128]
                )
                pw = ftpool.tile([128, GSZ, OS], BF16, tag="pw")
                nc.gpsimd.dma_start(
                    out=pw,
                    in_=pwtd[mh * PADMH + g0 * 128 : mh * PADMH + (g0 + GSZ) * 128, :]
                    .rearrange("(j p) o -> p j o", p=128),
                )
                opsum = psF.tile([64, OS], F32, tag="fps")
                for j in range(GSZ):
                    nc.tensor.matmul(
                        opsum, ft[:, j, :], pw[:, j, :],
                        start=(j == 0), stop=(j == GSZ - 1),
                    )
                nc.vector.tensor_add(oacc, oacc, opsum)

        def emit_gather(j):
            nc.gpsimd.collective_compute(
                "AllGather",
                OP.bypass,
                replica_groups=[list(range(NCORES))],
                ins=[feat8d[j][:]],
                outs=[featAG[j][:]],
            )

        # block order: (h, i) pairs with m inner so adjacent blocks use
        # disjoint PE row halves (m=0: rows 0-63, m=1: rows 64-127)
        blocks = []
        for h in range(H):
            for i in range(BP):
                for m in range(2):
                    blocks.append((m, h, i))
        NB = len(blocks)
        NP = NB // 2

        # per-block live state
        S_ps = {}     # (bi, nt) -> S psum tile
        E_t = {}      # (bi, nt) -> E sbuf bf16
        E2_t = {}     # (bi, nt) -> E^2 sbuf bf16
        ES_t = {}     # (bi, pair) -> E pair sums
        zr5 = {}      # bi -> [128, 8] f32 accum of exp row sums
        rzr5 = {}     # bi -> reciprocal
        zc_ps = {}    # bi -> zc psum [32, N]
        vpl_t = {}    # (bi, nt) -> vp scaled by 1/zr
        vpc_t = {}    # (bi, mc) -> vp scaled by 1/zc
        rzc_t = {}    # bi -> [128, 5] per-partition 1/zc
        up_ps = {}    # bi -> up psum [128, 5, 72]
        us_t = {}     # bi -> up evac sbuf
        fps_ps = {}   # bi -> fps psum [70, 72]

        proj_at = {3: [0, 1], 5: [2, 3]}
        gather_at = {1: [0, 1], 3: [2, 3], 4: [4]}

        def qk_mm(bi, nt):
            m, h, i = blocks[bi]
            qside, vside = 1 - m, m
            w = NT[nt]
            n0 = nt * 128
            pa = psS.tile([128, N], F32, tag="pq", name="pa")
            S_ps[bi, nt] = pa
            for c0, cw in NCH:
                nc.tensor.matmul(
                    pa[:w, c0 : c0 + cw],
                    qs[i, qside, h][:, n0 : n0 + w],
                    ks[i, vside, h][:, c0 : c0 + cw],
                    start=True, stop=True,
                )

        def qk_pair(ba, bb, nt):
            qk_mm(ba, nt)
            qk_mm(bb, nt)

        def exp_act(bi, nt):
            w = NT[nt]
            if nt == 0:
                zr5[bi] = zpool.tile([128, 8], F32, tag="zr5", name="zr5")
            et = epool.tile([128, N], BF16, tag="E", name="et")
            E_t[bi, nt] = et
            nc.scalar.activation(
                out=et[:w], in_=S_ps.pop((bi, nt))[:w], func=AX.Exp,
                accum_out=zr5[bi][:w, nt : nt + 1],
            )

        def e2_mul(bi, nt):
            w = NT[nt]
            e2 = e2pool.tile([128, N], BF16, tag="E2", name="e2")
            E2_t[bi, nt] = e2
            nc.vector.tensor_mul(e2[:w], E_t[bi, nt][:w], E_t[bi, nt][:w])

        def esum(bi):
            # full tree-reduce of the 5 E tiles to one [128, N] tile so the
            # zc colsum matmul streams 576 rows instead of 1728
            es0 = espool.tile([128, N], BF16, tag="es0", name="es0", bufs=2)
            nc.vector.tensor_add(es0, E_t[bi, 0], E_t[bi, 1])
            es1 = espool.tile([128, N], BF16, tag="es1", name="es1", bufs=2)
            nc.gpsimd.tensor_add(es1, E_t[bi, 2], E_t[bi, 3])
            es = espool.tile([128, N], BF16, tag="es2", name="es2")
            nc.vector.tensor_add(es, es0, es1)
            nc.vector.tensor_add(es[0:64], es[0:64], E_t[bi, 4][0:64])
            ES_t[bi] = es

        def rzr(bi):
            r = zpool.tile([128, 8], F32, tag="rzr5", name="rzr5")
            rzr5[bi] = r
            nc.vector.reciprocal(r[:, 0:5], zr5[bi][:, 0:5])

        def vpl_scale(bi, nt):
            m, h, i = blocks[bi]
            vside = m
            w = NT[nt]
            vpl = upool.tile([128, 72], BF16, tag="vpl", name="vpl")
            vpl_t[bi, nt] = vpl
            nc.vector.tensor_scalar_mul(
                vpl[:w, 0:70], vp[i, vside, nt][:w, h, 0:70],
                rzr5[bi][:w, nt : nt + 1],
            )

        def zc_mms(bi):
            zcp = psS.tile([128, N], F32, tag="pq", name="zcp")[0:32]
            zc_ps[bi] = zcp
            src_ = ES_t[bi]
            for c0, cw in NCH:
                nc.tensor.matmul(
                    zcp[:, c0 : c0 + cw], onesb_sb[:, :], src_[:, c0 : c0 + cw],
                    start=True, stop=True,
                )

        def rzc(bi):
            zt = tmp.tile([32, 18, 32], F32, tag="zt")
            nc.vector.transpose(
                out=zt, in_=zc_ps.pop(bi).rearrange("p (g q) -> p g q", q=32)
            )
            rz32 = zpool.tile([32, 18], F32, tag="rz32")
            nc.vector.reciprocal(rz32, zt[:, :, 0])
            r = zpool.tile([128, 5], F32, tag="rzc", name="rzc")
            rzc_t[bi] = r
            for a in range(4):
                ng = 5 if a < 2 else 4
                nc.vector.tensor_copy(
                    out=r[32 * a : 32 * a + 32, 0:ng],
                    in_=rz32[:, a : 18 : 4],
                )

        def vpc_scale(bi, mc):
            m, h, i = blocks[bi]
            vside = m
            w2 = NT[mc]
            vpc = upool.tile([128, 72], BF16, tag="vpc", name="vpc")
            vpc_t[bi, mc] = vpc
            nc.vector.tensor_scalar_mul(
                vpc[:w2, 0:70], vp[i, vside, mc][:w2, h, 0:70],
                rzc_t[bi][:w2, mc : mc + 1],
            )

        def up_mms(bi, nt):
            w = NT[nt]
            if nt == 0:
                up_ps[bi] = psU.tile([128, 5, 72], F32, tag="up", name="up")
            upt = up_ps[bi]
            e2 = E2_t[bi, nt]
            vpl = vpl_t[bi, nt]
            # single accumulation group for the whole bank: start=True clears
            # has_written for the entire 2KB bank, so per-mc groups would wipe
            # each other's partials. One group + per-element has_written works.
            for mc in range(5):
                w2 = NT[mc]
                nc.tensor.matmul(
                    upt[:w2, mc, 0:70],
                    e2[:w, mc * 128 : mc * 128 + w2],
                    vpl[:w, 0:70],
                    start=(nt == 0 and mc == 0), stop=(nt == 4 and mc == 4),
                    skip_group_check=True,
                )

        def up_evac(bi, on_act):
            us = uspool.tile([128, 5, 72], BF16, tag="us", name="us")
            us_t[bi] = us
            upt = up_ps.pop(bi)
            # read only matmul-written psum regions (mc 0-3 full, mc 4 rows 0-63)
            if on_act:
                nc.scalar.activation(out=us[:, 0:4, 0:70], in_=upt[:, 0:4, 0:70], func=AX.Copy)
                nc.scalar.activation(out=us[0:64, 4, 0:70], in_=upt[0:64, 4, 0:70], func=AX.Copy)
            else:
                nc.vector.tensor_copy(out=us[:, 0:4, 0:70], in_=upt[:, 0:4, 0:70])
                nc.vector.tensor_copy(out=us[0:64, 4, 0:70], in_=upt[0:64, 4, 0:70])
            # drop E2/vpl refs
            for nt in range(5):
                E2_t.pop((bi, nt))
                vpl_t.pop((bi, nt))

        def fps_mms(bi):
            fp = psF.tile([70, 72], F32, tag="fps", name="fps")
            fps_ps[bi] = fp
            us = us_t.pop(bi)
            for mc in range(5):
                w2 = NT[mc]
                nc.tensor.matmul(
                    fp[0:70, 0:70],
                    us[:w2, mc, 0:70],
                    vpc_t.pop((bi, mc))[:w2, 0:70],
                    start=(mc == 0), stop=(mc == 4),
                )

        def fb_out(bi):
            m, h, i = blocks[bi]
            mh = m * 3 + h
            fb = fpool.tile([70, 70], BF16, tag="fb", name="fb")
            nc.vector.tensor_copy(out=fb, in_=fps_ps.pop(bi)[0:70, 0:70])
            nc.sync.dma_start(
                out=feat8d[mh][i, 0:4900].rearrange("(d e) -> d e", e=70),
                in_=fb,
            )

        def drop_e(bi):
            for nt in range(5):
                E_t.pop((bi, nt))
            ES_t.pop(bi)
            zr5.pop(bi)
            rzr5.pop(bi)
            rzc_t.pop(bi)

        # prologue: QK + exp for pair 0 (blocks 0, 1), interleaved A/B
        for nt in range(5):
            qk_pair(0, 1, nt)
        for nt in range(5):
            exp_act(0, nt)
            exp_act(1, nt)

        # gather j after the fb of its (h, i=7) pair lands (pair-iter 8h+7+2);
        # proj j a few pair-iters after its gather
        gather_sched = {10: [0, 3], 18: [1, 4]}
        proj_sched = {13: [0], 16: [3], 21: [1], 24: [4]}

        # steady state over pairs: iteration p emits QK(p), math(p-1), out(p-2)
        evac_q = []
        for p in range(1, NP + 3):
            A, Bb = 2 * p, 2 * p + 1          # current pair blocks
            A1, B1 = A - 2, Bb - 2            # math stage
            A2, B2 = A - 6, Bb - 6            # out stage (after up_evac lag)
            for pj in proj_sched.get(p, []):
                emit_proj(pj)
            # DVE: evacuate previous pair's up psums first (frees psU bank),
            # then the math-front for (A1, B1)
            for X in evac_q:
                up_evac(X, on_act=False)
                for mc in range(5):
                    vpc_scale(X, mc)
            evac_q = []
            if A1 < NB:
                for X in (A1, B1):
                    rzr(X)
                    for nt in range(5):
                        e2_mul(X, nt)
                        vpl_scale(X, nt)
                    esum(X)
            # PE: QK pairs interleaved with fps of the out stage
            if p < NP:
                qk_pair(A, Bb, 0)
            if 0 <= A2 < NB:
                fps_mms(A2)
            if p < NP:
                qk_pair(A, Bb, 1)
            if 0 <= B2 < NB:
                fps_mms(B2)
            if p < NP:
                for nt in range(2, 5):
                    qk_pair(A, Bb, nt)
            if A1 < NB:
                for nt in range(5):
                    up_mms(A1, nt)
                zc_mms(A1)
                for nt in range(5):
                    up_mms(B1, nt)
                zc_mms(B1)
                evac_q = [A1, B1]
                    # ACT: exps for pair p in S-pool allocation order
            if p < NP:
                for nt in range(5):
                    exp_act(A, nt)
                    exp_act(Bb, nt)
            if A1 < NB:
                rzc(A1)
                rzc(B1)
            if 0 <= A2 < NB:
                fb_out(A2)
                fb_out(B2)
                drop_e(A2)
                drop_e(B2)
            for j in gather_sched.get(p, []):
                emit_gather(j)

        emit_gather(2)
        emit_gather(5)
        emit_proj(2)
        emit_proj(5)
        osb = opool.tile([B, OS], F32, tag="osb")
        nc.vector.tensor_add(osb, oacc, pb_sb)
        nc.vector.tensor_scalar_max(osb, osb, 0.0)
        nc.sync.dma_start(out=outd[:], in_=osb)

    nc.compile()
    return nc


def kernel(x1, x2, ln_w, ln_b, qkv_w, proj_w, proj_b):
    wqk, wv, pos_pad, pwt, tqkT, tv = _host_prep(ln_w, ln_b, qkv_w, proj_w, proj_b)
    xs1 = np.ascontiguousarray(x1.reshape(B, C, N)).astype(ml_dtypes.bfloat16)
    xs2 = np.ascontiguousarray(x2.reshape(B, C, N)).astype(ml_dtypes.bfloat16)
    nc = _build()
    in_maps = []
    for r in range(NCORES):
        in_maps.append(
            {
                "x1s": xs1[r * BP : (r + 1) * BP],
                "x2s": xs2[r * BP : (r + 1) * BP],
                "wqk": wqk,
                "wv": wv,
                "pos": pos_pad,
                "pwt": np.ascontiguousarray(pwt[:, r * OS : (r + 1) * OS]),
                "pb": np.ascontiguousarray(proj_b[None, r * OS : (r + 1) * OS]).astype(np.float32),
            }
        )
    import os

    trace = bool(os.environ.get("BASS_TRACE"))
    res = run_bass_kernel_spmd(nc, in_maps, core_ids=list(range(NCORES)), trace=trace)
    if res.exec_time_ns is not None:
        print(f"HW exec time: {res.exec_time_ns} ns")
    if res.instructions_and_trace:
        print("trace path:", res.instructions_and_trace[1])
    out = np.concatenate([res.results[r]["out"] for r in range(NCORES)], axis=1)
    return out.astype(np.float32)


if __name__ == "__main__":
    rng = np.random.default_rng(0)
    ins = {
        "x1": rng.standard_normal((B, C, HG, WG), dtype=np.float32),
        "x2": rng.standard_normal((B, C, HG, WG), dtype=np.float32),
        "ln_w": np.ones(C, np.float32),
        "ln_b": np.zeros(C, np.float32),
        "qkv_w": (rng.standard_normal((3 * F, C)) * C**-0.5).astype(np.float32),
        "proj_w": (rng.standard_normal((512, 6 * 4900)) * (6 * 4900) ** -0.5).astype(np.float32),
        "proj_b": np.zeros(512, np.float32),
    }
    print(kernel(**ins).shape)
